# revision 1
# baseline (speedup 1.0000x reference)
"""MoE feed-forward (top-2 of 8 experts) on 8 Trainium2 NeuronCores.

Strategy: expert-parallel with load balancing. Each of the 8 cores owns one
expert's weights (its "primary" set) plus optionally a second expert's
weights (its "secondary" set). The (tiny) gate runs on host as part of input
sharding: top-2 routing is computed in float64 (ordering verified robust:
min weight gap between rank-2/rank-3 experts is ~6.6e-6, far above f32
rounding noise). Tokens are gathered per expert; each expert's first C_A
tokens go to its own core, and overflow tokens (experts loaded above C_A)
are packed into 128-token blocks dispatched to cores with spare capacity,
which receive that expert's weights as their secondary set. This keeps all
FLOPs on device while bounding every core's token count by C_A + C_B
instead of max_e count_e.

Each core computes, in bf16 with f32 PSUM accumulation,

    out_row = (silu(x_row @ W1[set]) @ W2[set]) * combine_weight_row

for its gathered tokens. The host then un-shards: every token's output is
the sum of its two expert rows (combine weights already applied on device).
"""

import numpy as np
import ml_dtypes

B, T, D, H, E = 4, 2048, 1024, 2048, 8
TOP_K = 2
N = B * T
P = 128
NCORES = 8
MM_FREE = 512  # PSUM bank-limited matmul free dim (fp32 out)

_compiled = {}


def _chunks(lo, hi, step):
    out = []
    while lo < hi:
        out.append((lo, min(step, hi - lo)))
        lo += min(step, hi - lo)
    return out


def _build(C_A, C_B):
    """Compile the per-core program: C_A primary-expert tokens followed by
    C_B secondary-expert tokens (C_B may be 0)."""
    import concourse.bacc as bacc
    import concourse.mybir as mybir
    import concourse.tile as tile

    fp32 = mybir.dt.float32
    bf16 = mybir.dt.bfloat16

    C = C_A + C_B
    n_sets = 2 if C_B else 1

    nc = bacc.Bacc("TRN2", target_bir_lowering=False, debug=False)

    xT = nc.dram_tensor("xT", [D, C], bf16, kind="ExternalInput").ap()
    w1d = [nc.dram_tensor(f"w1{s}", [D, H], bf16, kind="ExternalInput").ap()
           for s in range(n_sets)]
    w2d = [nc.dram_tensor(f"w2{s}", [H, D], bf16, kind="ExternalInput").ap()
           for s in range(n_sets)]
    wt = nc.dram_tensor("wt", [P, C // P], fp32, kind="ExternalInput").ap()
    out = nc.dram_tensor("out", [C, D], fp32, kind="ExternalOutput").ap()

    KD = D // P   # contraction tiles for x @ W1
    KH = H // P   # contraction tiles for h @ W2
    NJ = D // MM_FREE
    NW1C = H // MM_FREE  # w1 column chunks

    # token groups never straddle the primary/secondary boundary
    groups = [(g0, gs, 0) for g0, gs in _chunks(0, C_A, MM_FREE)]
    groups += [(g0, gs, 1) for g0, gs in _chunks(C_A, C, MM_FREE)]

    with tile.TileContext(nc) as tc:
        with (
            tc.tile_pool(name="persist", bufs=1) as persist,
            tc.tile_pool(name="hpool", bufs=2 * KH + 2) as hpool,
            tc.tile_pool(name="opool", bufs=4) as opool,
            tc.tile_pool(name="psum1", bufs=4, space="PSUM") as psum1,
            tc.tile_pool(name="psum2", bufs=4, space="PSUM") as psum2,
        ):
            # DMA inputs in PE consumption order, chunked to 512 columns so
            # the first token group's operands land within a few µs:
            #   wt, xT[g0], w1a (c0..c3), w2a, xT[g1..], w1b, w2b
            # Descriptor issue costs ~0.6µs of sequencer time per dma_start,
            # so issues round-robin across three otherwise-idle sequencers
            # instead of serializing on Sync.
            # Only the 16 critical ramp chunks split across the two HWDGE
            # engines (Sync + Scalar); everything else stays on Sync. GpSimd's
            # SWDGE path and bulk issues on Scalar both measured slower.
            crit = [False]

            def in_dma(out_, in_):
                eng = nc.scalar if crit[0] else nc.sync
                eng.dma_start(out=out_, in_=in_)

            def out_dma(out_, in_):
                nc.sync.dma_start(out=out_, in_=in_)


            xT_sb = [[None] * len(groups) for _ in range(KD)]

            def load_xT_chunk(gi, k):
                g0, gs, _ = groups[gi]
                tx = persist.tile(
                    [P, gs], bf16, tag=f"xT_{k}_{gi}", name=f"xT_{k}_{gi}"
                )
                in_dma(tx, xT[k * P:(k + 1) * P, g0:g0 + gs])
                xT_sb[k][gi] = tx

            def load_xT_group(gi):
                for k in range(KD):
                    load_xT_chunk(gi, k)

            w1_sb = [[[None] * NW1C for _ in range(KD)] for _ in range(n_sets)]
            w2_sb = [[None] * KH for _ in range(n_sets)]

            def load_w1_chunk(s, k, c):
                t1 = persist.tile([P, MM_FREE], bf16, tag=f"w1_{s}_{k}_{c}",
                                  name=f"w1_{s}_{k}_{c}")
                in_dma(t1, w1d[s][k * P:(k + 1) * P, c * MM_FREE:(c + 1) * MM_FREE])
                w1_sb[s][k][c] = t1

            def load_w1(s):
                for c in range(NW1C):
                    for k in range(KD):
                        load_w1_chunk(s, k, c)

            def load_w2_chunk(s, i):
                t2 = persist.tile([P, D], bf16, tag=f"w2_{s}_{i}",
                                  name=f"w2_{s}_{i}")
                in_dma(t2, w2d[s][i * P:(i + 1) * P, :])
                w2_sb[s][i] = t2

            def load_w2(s):
                for i in range(KH):
                    load_w2_chunk(s, i)

            # critical path first: k-pairs of (xT g0, w1 c0), then the rest of
            # w1, then w2 interleaved with the remaining xT groups.
            for k in range(KD):
                crit[0] = True
                load_xT_chunk(0, k)
                crit[0] = False
                load_w1_chunk(0, k, 0)
            wt_sb = persist.tile([P, C // P], fp32, tag="wt", name="wt_sb")
            in_dma(wt_sb, wt[:, :])
            for c in range(1, NW1C):
                for k in range(KD):
                    load_w1_chunk(0, k, c)
            for i in range(KH):
                load_w2_chunk(0, i)
                gi = 1 + i // 2
                k = (i % 2) * (KD // 2)
                if gi < len(groups):
                    for kk in range(k, k + KD // 2):
                        load_xT_chunk(gi, kk)
            for gi in range(1 + KH // 2, len(groups)):
                load_xT_group(gi)
            if n_sets > 1:
                load_w1(1)
                load_w2(1)

            # PE warm-up: dummy matmuls on an uninitialized tile while the
            # first operand DMAs are in flight (HAM un-throttles after ~3.4µs
            # of activity, so the real matmuls start at 2.4 GHz).

            # Software-pipelined group schedule: mm1(g0), mm1(g1), mm2(g0),
            # mm1(g2), mm2(g1), ... — the PE always has independent work at
            # every mm1→mm2 boundary (mm2(g) needs all KH hT tiles of g, so
            # issuing mm1(g+1) in between hides the silu tail and any w2
            # delivery lag without ever idling the PE).
            def mm1(gi):
                g0, gs, s = groups[gi]
                hts = []
                for i in range(KH):
                    ci, co = divmod(i * P, MM_FREE)
                    ps = psum1.tile([P, MM_FREE], fp32, tag="ps1", name=f"ps1_{g0}_{i}")
                    for k in range(KD):
                        nc.tensor.matmul(
                            ps[:, :gs],
                            w1_sb[s][k][ci][:, co:co + P],
                            xT_sb[k][gi],
                            start=(k == 0),
                            stop=(k == KD - 1),
                        )
                    ht = hpool.tile([P, MM_FREE], bf16, tag="hT", name=f"hT_{g0}_{i}")
                    nc.scalar.activation(
                        ht[:, :gs], ps[:, :gs], mybir.ActivationFunctionType.Silu
                    )
                    hts.append(ht)
                return hts

            def mm2(gi, hts):
                g0, gs, s = groups[gi]
                for t in range(gs // P):
                    tok = g0 + t * P
                    for j in range(NJ):
                        ps2 = psum2.tile(
                            [P, MM_FREE], fp32, tag="ps2", name=f"ps2_{tok}_{j}"
                        )
                        for i in range(KH):
                            nc.tensor.matmul(
                                ps2,
                                hts[i][:, t * P:(t + 1) * P],
                                w2_sb[s][i][:, j * MM_FREE:(j + 1) * MM_FREE],
                                start=(i == 0),
                                stop=(i == KH - 1),
                            )
                        ot = opool.tile([P, MM_FREE], fp32, tag="ot", name=f"ot_{tok}_{j}")
                        nc.vector.tensor_scalar_mul(
                            ot, ps2, wt_sb[:, tok // P: tok // P + 1]
                        )
                        out_dma(out[tok:tok + P, j * MM_FREE:(j + 1) * MM_FREE], ot)

            prev = (0, mm1(0))
            for gi in range(1, len(groups)):
                hts = mm1(gi)
                mm2(*prev)
                prev = (gi, hts)
            mm2(*prev)

    nc.compile()
    return nc


def _get_compiled(C_A, C_B):
    key = (C_A, C_B)
    if key not in _compiled:
        _compiled[key] = _build(C_A, C_B)
    return _compiled[key]


def _plan_capacity(counts):
    """Pick (C_A, C_B): the smallest 128-aligned primary capacity whose
    overflow fits in one 128-token secondary block per core."""
    mean_cap = int(-(-counts.sum() // (NCORES * P)) * P)
    max_cap = int(-(-counts.max() // P) * P)
    for C_A in range(mean_cap, max_cap + P, P):
        over = np.maximum(counts - C_A, 0)
        nblocks = int(np.sum(-(-over // P)))
        if nblocks == 0:
            return C_A, 0
        if nblocks <= NCORES:
            return C_A, P
    return max_cap, 0


def kernel(**inputs):
    x = np.asarray(inputs["x"], dtype=np.float32)
    Wg = np.asarray(inputs["Wg"], dtype=np.float32)
    W1 = np.asarray(inputs["W1"], dtype=np.float32)
    W2 = np.asarray(inputs["W2"], dtype=np.float32)
    xf = np.ascontiguousarray(x.reshape(-1, D))

    # --- host-side gate + top-2 routing (float64; ordering matches f32 ref) ---
    logits = xf.astype(np.float64) @ Wg.astype(np.float64)
    w = np.exp(logits - logits.max(axis=-1, keepdims=True))
    w /= w.sum(axis=-1, keepdims=True)
    order = np.argsort(-w, axis=-1, kind="stable")[:, :TOP_K]  # [N, 2] expert ids
    tw = np.take_along_axis(w, order, axis=-1)
    tw = tw / tw.sum(axis=-1, keepdims=True)  # renormalized combine weights

    counts = np.bincount(order.ravel(), minlength=E)
    C_A, C_B = _plan_capacity(counts)
    C = C_A + C_B

    nc = _get_compiled(C_A, C_B)

    # --- dispatch: primary segment per expert-owner core + overflow blocks ---
    bf = ml_dtypes.bfloat16
    tok_of = []    # per expert: token ids routed to it (ascending)
    wt_of = []     # matching combine weights
    for e in range(E):
        sel = np.nonzero((order == e).any(axis=-1))[0]
        slot = (order[sel, 1] == e).astype(np.int64)
        tok_of.append(sel)
        wt_of.append(tw[sel, slot].astype(np.float32))

    # overflow blocks (expert, token ids, weights), ≤128 tokens each
    blocks = []
    for e in range(E):
        for b0 in range(C_A, len(tok_of[e]), P):
            blocks.append((e, tok_of[e][b0:b0 + P], wt_of[e][b0:b0 + P]))
    assert len(blocks) <= NCORES, (counts, C_A, C_B)

    pos = np.empty((N, TOP_K), dtype=np.int64)
    in_maps = []
    for c in range(NCORES):
        prim_tok = tok_of[c][:C_A]
        prim_wt = wt_of[c][:C_A]
        slot = (order[prim_tok, 1] == c).astype(np.int64)
        pos[prim_tok, slot] = c * C + np.arange(len(prim_tok))

        xTe = np.zeros((D, C), dtype=bf)
        xTe[:, :len(prim_tok)] = xf[prim_tok].T.astype(bf)
        wtp = np.zeros(C, dtype=np.float32)
        wtp[:len(prim_tok)] = prim_wt

        m = {
            "xT": xTe,
            "w10": np.ascontiguousarray(W1[c]).astype(bf),
            "w20": np.ascontiguousarray(W2[c]).astype(bf),
        }
        if C_B:
            if c < len(blocks):
                be, btok, bwt = blocks[c]
                xTe[:, C_A:C_A + len(btok)] = xf[btok].T.astype(bf)
                wtp[C_A:C_A + len(btok)] = bwt
                bslot = (order[btok, 1] == be).astype(np.int64)
                pos[btok, bslot] = c * C + C_A + np.arange(len(btok))
                m["w11"] = np.ascontiguousarray(W1[be]).astype(bf)
                m["w21"] = np.ascontiguousarray(W2[be]).astype(bf)
            else:
                m["w11"] = np.zeros((D, H), dtype=bf)
                m["w21"] = np.zeros((H, D), dtype=bf)
        m["wt"] = np.ascontiguousarray(wtp.reshape(C // P, P).T)
        in_maps.append(m)

    from concourse.bass_utils import run_bass_kernel_spmd

    # The SPMD launch reaches the 8 NeuronCores through jax/PJRT. If the
    # calling process pinned jax to CPU (e.g. to run the reference), flip to
    # the axon platform for the launch and restore afterwards.
    import jax

    flipped = False
    try:
        n_acc = len([d for d in jax.devices() if d.platform != "cpu"])
    except Exception:
        n_acc = 0
    def _clear_backends():
        try:
            import jax.extend.backend as jeb
            jeb.clear_backends()
        except Exception:
            from jax._src import xla_bridge
            xla_bridge._clear_backends()

    if n_acc < NCORES:
        prev = jax.config.jax_platforms
        jax.config.update("jax_platforms", "axon")
        _clear_backends()
        flipped = True
    try:
        res = run_bass_kernel_spmd(nc, in_maps, core_ids=list(range(NCORES)))
    finally:
        if flipped:
            jax.config.update("jax_platforms", prev)
            _clear_backends()

    Y = np.concatenate([res.results[c]["out"] for c in range(NCORES)], axis=0)
    outf = Y[pos[:, 0]] + Y[pos[:, 1]]
    return outf.reshape(B, T, D).astype(np.float32)



# revision 9
# speedup vs baseline: 1.0025x; 1.0025x over previous
"""MoE feed-forward (top-2 of 8 experts) on 8 Trainium2 NeuronCores.

Strategy: expert-parallel, primary-only. Core e owns expert e's weights and
processes every token routed to expert e (capacity C = ceil128(max expert
count); for the graded distribution max count 2151 -> C = 2176, the same
device cost as the old primary+secondary scheme but with half the weight
DMA and a simpler dispatch). The (tiny) gate runs on host in float64
(ordering robust: min rank-2/rank-3 weight gap ~6.6e-6 >> f32 noise).

Each core computes, in bf16 with f32 PSUM accumulation,

    out_row = (silu(x_row @ W1[e]) @ W2[e]) * combine_weight_row

for its tokens (zero-padded slots get combine weight 0). Host un-shards:
every token's output is the sum of its two expert rows.

Perf notes (from perfetto traces of the previous version):
 - ~10us of head latency is framework preamble (6.6us) + first-operand DMA.
   Inputs are host-packed into SBUF-image DRAM layouts so the whole input
   set needs ~15 descriptor issues (vs ~150 at 0.6us each), and group 0's
   x/W1 chunks stream in 256-512KB bundles consumed k-phase by k-phase, so
   the PE never starves more than ~0.5us at the start.
 - HAM clock-gate: the PE runs at 1.2 GHz until ~3.4us of sustained
   activity. Dummy warm-up matmuls on a memset tile run during the
   preamble/DMA window so real matmuls start at 2.4 GHz.
 - mm1 groups must keep moving-dim >= 256: a 128-token group is
   LDWEIGHTS-bound (~81ns/MM for 53ns of streaming). Token groups are
   [512,512,512,384,256] instead of [512x4,128].
"""

import numpy as np
import ml_dtypes

B, T, D, H, E = 4, 2048, 1024, 2048, 8
TOP_K = 2
N = B * T
P = 128
NCORES = 8
MM_FREE = 512  # PSUM bank-limited matmul free dim (fp32 out)
KD = D // P    # contraction tiles for x @ W1
KH = H // P    # contraction tiles for h @ W2
NJ = D // MM_FREE
NW1C = H // MM_FREE  # w1 column chunks

_compiled = {}


def _plan_groups(C):
    """Split C (multiple of 128) into mm1 token groups, each a multiple of
    128 with 256 <= size <= 512 (small moving dims are LDWEIGHTS-bound)."""
    assert C % P == 0 and C >= 512
    groups = []
    rem = C
    while rem > 768:
        groups.append(512)
        rem -= 512
    if rem > 512:  # 640 or 768
        groups.append(rem - 256)
        rem = 256
    groups.append(rem)
    return groups


def _build(C):
    import concourse.bacc as bacc
    import concourse.mybir as mybir
    import concourse.tile as tile

    fp32 = mybir.dt.float32
    bf16 = mybir.dt.bfloat16

    gsizes = _plan_groups(C)
    g0 = gsizes[0]          # first group (512 unless C < 512+256)
    Cr = C - g0             # tokens in the remaining groups
    # token offset of each group
    goff = [0]
    for gs in gsizes:
        goff.append(goff[-1] + gs)

    nc = bacc.Bacc("TRN2", target_bir_lowering=False, debug=False)

    # SBUF-image DRAM layouts (one DMA lands exactly one SBUF tile image):
    #   xg0  [P, KD*g0]   x^T chunks k=0..7 of group 0, k-major
    #   xg12 [P, KD*n12]  x^T chunks of groups 1..2, k-major
    #   xg34 [P, KD*n34]  x^T chunks of groups 3..., k-major
    #   w1c{c} [P, KD*MM_FREE]  W1 column block c, k-major
    #   w2i  [P, KH*D]    W2 row chunks i=0..15
    #   wt   [P, C//P]    combine weights, token t at [t%P, t//P]
    n12 = sum(gsizes[1:3]) if len(gsizes) > 1 else 0
    n34 = Cr - n12
    xg0_d = nc.dram_tensor("xg0", [P, KD * g0], bf16, kind="ExternalInput").ap()
    xg12_d = xg34_d = None
    if n12:
        xg12_d = nc.dram_tensor("xg12", [P, KD * n12], bf16, kind="ExternalInput").ap()
    if n34:
        xg34_d = nc.dram_tensor("xg34", [P, KD * n34], bf16, kind="ExternalInput").ap()
    w1c_d = [nc.dram_tensor(f"w1c{c}", [P, KD * MM_FREE], bf16, kind="ExternalInput").ap()
             for c in range(NW1C)]
    w2_d = nc.dram_tensor("w2i", [P, KH * D], bf16, kind="ExternalInput").ap()
    wt_d = nc.dram_tensor("wt", [P, C // P], fp32, kind="ExternalInput").ap()
    out = nc.dram_tensor("out", [C, D], fp32, kind="ExternalOutput").ap()

    with tile.TileContext(nc) as tc:
        with (
            tc.tile_pool(name="persist", bufs=1) as persist,
            tc.tile_pool(name="hpool", bufs=2 * KH + 2) as hpool,
            tc.tile_pool(name="opool", bufs=4) as opool,
            tc.tile_pool(name="psum1", bufs=4, space="PSUM") as psum1,
            tc.tile_pool(name="psum2", bufs=4, space="PSUM") as psum2,
        ):
            # --- PE warm-up: HAM un-throttles after ~3.4us of activity; run
            # dummy matmuls on a memset tile so real matmuls start at 2.4GHz.
            warm = persist.tile([P, MM_FREE], bf16, tag="warm", name="warm")
            nc.gpsimd.memset(warm[:, :], 0.0)
            for r in range(9):
                wps = psum2.tile([P, MM_FREE], fp32, tag="ps2", name=f"warm_{r}")
                nc.tensor.matmul(wps, warm[:, :P], warm, start=True, stop=True)

            # --- input DMA: group-0 operands stream in k-pair bundles that
            # the k-phased first i-sweep consumes as they land; later tensors
            # are whole-image issues ordered by first use. All on the sync
            # queue except xg0 (scalar) so the two first-needed tensors
            # transfer concurrently.
            xg0_sb = []          # 4 tiles, k-pair bundles of group 0
            for p in range(KD // 2):
                t = persist.tile([P, 2 * g0], bf16, tag=f"xg0_{p}", name=f"xg0_{p}")
                nc.scalar.dma_start(out=t, in_=xg0_d[:, 2 * p * g0:(2 * p + 2) * g0])
                xg0_sb.append(t)

            # per-column-block bundle tiles (separate tiles keep DMA
            # dependencies per-bundle so k-phased consumers start early)
            w1_parts = {0: 4, 1: 4, 2: 2, 3: 2}  # bundle counts per block
            w1_sb = [None] * NW1C  # list of (bundle tiles, ks-per-bundle)

            def load_w1(c):
                npart = w1_parts.get(c, 1)
                kper = KD // npart
                w = kper * MM_FREE
                tiles = []
                for p in range(npart):
                    t = persist.tile([P, w], bf16, tag=f"w1_{c}_{p}",
                                     name=f"w1_{c}_{p}")
                    nc.sync.dma_start(out=t, in_=w1c_d[c][:, p * w:(p + 1) * w])
                    tiles.append(t)
                w1_sb[c] = (tiles, kper)

            # c0 bundles must land first; then c1..c3, then x of later
            # groups, then w2 (first needed at first mm2), then wt.
            for c in range(NW1C):
                load_w1(c)
            xg12_sb = xg34_sb = None
            if n12:
                xg12_sb = persist.tile([P, KD * n12], bf16, tag="xg12", name="xg12")
                nc.sync.dma_start(out=xg12_sb, in_=xg12_d[:, :])
            if n34:
                xg34_sb = persist.tile([P, KD * n34], bf16, tag="xg34", name="xg34")
                nc.sync.dma_start(out=xg34_sb, in_=xg34_d[:, :])
            w2_sb = []
            for p in range(2):
                w = KH * D // 2
                t = persist.tile([P, w], bf16, tag=f"w2_{p}", name=f"w2_{p}")
                nc.sync.dma_start(out=t, in_=w2_d[:, p * w:(p + 1) * w])
                w2_sb.append(t)
            wt_sb = persist.tile([P, C // P], fp32, tag="wt", name="wt_sb")
            nc.sync.dma_start(out=wt_sb, in_=wt_d[:, :])

            def xT_view(k, gi):
                """[P, gsizes[gi]] view of x^T chunk k for group gi."""
                t0, gs = goff[gi], gsizes[gi]
                if gi == 0:
                    return xg0_sb[k // 2][:, (k % 2) * g0:(k % 2) * g0 + g0]
                if gi <= 2 and n12:
                    o = t0 - g0
                    return xg12_sb[:, k * n12 + o:k * n12 + o + gs]
                o = t0 - g0 - n12
                return xg34_sb[:, k * n34 + o:k * n34 + o + gs]

            def w1_view(k, c, co):
                """[P, P] stationary: W1 rows k*128.., cols c*512+co."""
                tiles, kper = w1_sb[c]
                o = (k % kper) * MM_FREE + co
                return tiles[k // kper][:, o:o + P]

            def w2_view(i, j):
                """[P, MM_FREE]: W2 rows i*128.., cols j*512.."""
                kper = KH // 2
                o = (i % kper) * D + j * MM_FREE
                return w2_sb[i // kper][:, o:o + MM_FREE]

            # --- software-pipelined groups: mm1(g0), mm1(g1), mm2(g0), ...
            def mm1(gi):
                gs = gsizes[gi]
                hts = []
                if gi == 0:
                    # k-phased first i-sweep: i=0..3 accumulate k-pairs as the
                    # xg0/w1c0 bundles land (start only needs 512KB in SBUF).
                    pss = [psum1.tile([P, MM_FREE], fp32, tag="ps1", name=f"ps1_0_{i}")
                           for i in range(4)]
                    for kp in range(KD // 2):
                        for i in range(4):
                            for k in (2 * kp, 2 * kp + 1):
                                nc.tensor.matmul(
                                    pss[i][:, :gs], w1_view(k, 0, i * P),
                                    xT_view(k, 0),
                                    start=(k == 0), stop=(k == KD - 1),
                                )
                    for i in range(4):
                        ht = hpool.tile([P, MM_FREE], bf16, tag="hT", name=f"hT_0_{i}")
                        nc.scalar.activation(
                            ht[:, :gs], pss[i][:, :gs],
                            mybir.ActivationFunctionType.Silu,
                        )
                        hts.append(ht)
                    irange = range(4, KH)
                else:
                    irange = range(KH)
                for i in irange:
                    ci, co = divmod(i * P, MM_FREE)
                    ps = psum1.tile([P, MM_FREE], fp32, tag="ps1", name=f"ps1_{gi}_{i}")
                    for k in range(KD):
                        nc.tensor.matmul(
                            ps[:, :gs], w1_view(k, ci, co), xT_view(k, gi),
                            start=(k == 0), stop=(k == KD - 1),
                        )
                    ht = hpool.tile([P, MM_FREE], bf16, tag="hT", name=f"hT_{gi}_{i}")
                    nc.scalar.activation(
                        ht[:, :gs], ps[:, :gs], mybir.ActivationFunctionType.Silu
                    )
                    hts.append(ht)
                return hts

            def mm2(gi, hts):
                t0, gs = goff[gi], gsizes[gi]
                for t in range(gs // P):
                    tok = t0 + t * P
                    for j in range(NJ):
                        ps2 = psum2.tile([P, MM_FREE], fp32, tag="ps2",
                                         name=f"ps2_{tok}_{j}")
                        for i in range(KH):
                            nc.tensor.matmul(
                                ps2,
                                hts[i][:, t * P:(t + 1) * P],
                                w2_view(i, j),
                                start=(i == 0), stop=(i == KH - 1),
                            )
                        ot = opool.tile([P, MM_FREE], fp32, tag="ot",
                                        name=f"ot_{tok}_{j}")
                        nc.vector.tensor_scalar_mul(
                            ot, ps2, wt_sb[:, tok // P:tok // P + 1]
                        )
                        nc.sync.dma_start(
                            out=out[tok:tok + P, j * MM_FREE:(j + 1) * MM_FREE], in_=ot
                        )

            prev = (0, mm1(0))
            for gi in range(1, len(gsizes)):
                hts = mm1(gi)
                mm2(*prev)
                prev = (gi, hts)
            mm2(*prev)

    nc.compile()
    return nc


def _get_compiled(C):
    if C not in _compiled:
        _compiled[C] = _build(C)
    return _compiled[C]


def _pack_xT(xTe, g0, n12, n34):
    """Split x^T [D, C] into the k-major SBUF-image layouts."""
    arr = np.ascontiguousarray(xTe).reshape(KD, P, xTe.shape[1])
    m = {"xg0": np.ascontiguousarray(
        arr[:, :, :g0].transpose(1, 0, 2).reshape(P, KD * g0))}
    if n12:
        m["xg12"] = np.ascontiguousarray(
            arr[:, :, g0:g0 + n12].transpose(1, 0, 2).reshape(P, KD * n12))
    if n34:
        m["xg34"] = np.ascontiguousarray(
            arr[:, :, g0 + n12:].transpose(1, 0, 2).reshape(P, KD * n34))
    return m


def kernel(**inputs):
    x = np.asarray(inputs["x"], dtype=np.float32)
    Wg = np.asarray(inputs["Wg"], dtype=np.float32)
    W1 = np.asarray(inputs["W1"], dtype=np.float32)
    W2 = np.asarray(inputs["W2"], dtype=np.float32)
    xf = np.ascontiguousarray(x.reshape(-1, D))

    # --- host-side gate + top-2 routing (float64; ordering matches f32 ref) ---
    logits = xf.astype(np.float64) @ Wg.astype(np.float64)
    w = np.exp(logits - logits.max(axis=-1, keepdims=True))
    w /= w.sum(axis=-1, keepdims=True)
    order = np.argsort(-w, axis=-1, kind="stable")[:, :TOP_K]  # [N, 2] expert ids
    tw = np.take_along_axis(w, order, axis=-1)
    tw = tw / tw.sum(axis=-1, keepdims=True)  # renormalized combine weights

    counts = np.bincount(order.ravel(), minlength=E)
    C = int(-(-max(int(counts.max()), 512) // P) * P)
    # per-partition SBUF: xT images 16*C bytes + ~107KB of weights/pools
    assert 16 * C + 110 * 1024 < 200 * 1024, "pathological routing skew"

    nc = _get_compiled(C)
    gsizes = _plan_groups(C)
    g0 = gsizes[0]
    n12 = sum(gsizes[1:3]) if len(gsizes) > 1 else 0
    n34 = C - g0 - n12

    bf = ml_dtypes.bfloat16
    pos = np.empty((N, TOP_K), dtype=np.int64)
    in_maps = []
    for e in range(E):
        sel = np.nonzero((order == e).any(axis=-1))[0]
        slot = (order[sel, 1] == e).astype(np.int64)
        pos[sel, slot] = e * C + np.arange(len(sel))

        xTe = np.zeros((D, C), dtype=bf)
        xTe[:, :len(sel)] = xf[sel].T.astype(bf)
        wtp = np.zeros(C, dtype=np.float32)
        wtp[:len(sel)] = tw[sel, slot].astype(np.float32)

        m = _pack_xT(xTe, g0, n12, n34)
        W1e = np.ascontiguousarray(W1[e]).astype(bf).reshape(KD, P, NW1C, MM_FREE)
        for c in range(NW1C):
            m[f"w1c{c}"] = np.ascontiguousarray(
                W1e[:, :, c, :].transpose(1, 0, 2).reshape(P, KD * MM_FREE))
        m["w2i"] = np.ascontiguousarray(
            np.ascontiguousarray(W2[e]).astype(bf).reshape(KH, P, D)
            .transpose(1, 0, 2).reshape(P, KH * D))
        m["wt"] = np.ascontiguousarray(wtp.reshape(C // P, P).T)
        in_maps.append(m)

    from concourse.bass_utils import run_bass_kernel_spmd

    # The SPMD launch reaches the 8 NeuronCores through jax/PJRT. If the
    # calling process pinned jax to CPU (e.g. to run the reference), flip to
    # the axon platform for the launch and restore afterwards.
    import jax

    flipped = False
    try:
        n_acc = len([d for d in jax.devices() if d.platform != "cpu"])
    except Exception:
        n_acc = 0

    def _clear_backends():
        try:
            import jax.extend.backend as jeb
            jeb.clear_backends()
        except Exception:
            from jax._src import xla_bridge
            xla_bridge._clear_backends()

    if n_acc < NCORES:
        prev = jax.config.jax_platforms
        jax.config.update("jax_platforms", "axon")
        _clear_backends()
        flipped = True
    try:
        res = run_bass_kernel_spmd(nc, in_maps, core_ids=list(range(NCORES)))
    finally:
        if flipped:
            jax.config.update("jax_platforms", prev)
            _clear_backends()

    Y = np.concatenate([res.results[c]["out"] for c in range(NCORES)], axis=0)
    outf = Y[pos[:, 0]] + Y[pos[:, 1]]
    return outf.reshape(B, T, D).astype(np.float32)


# revision 10
# speedup vs baseline: 1.0074x; 1.0049x over previous
"""MoE feed-forward (top-2 of 8 experts) on 8 Trainium2 NeuronCores.

Strategy: expert-parallel, primary-only. Core e owns expert e's weights and
processes every token routed to expert e (capacity C = ceil128(max expert
count); for the graded distribution max count 2151 -> C = 2176, the same
device cost as the old primary+secondary scheme but with half the weight
DMA and a simpler dispatch). The (tiny) gate runs on host in float64
(ordering robust: min rank-2/rank-3 weight gap ~6.6e-6 >> f32 noise).

Each core computes, in bf16 with f32 PSUM accumulation,

    out_row = (silu(x_row @ W1[e]) @ W2[e]) * combine_weight_row

for its tokens (zero-padded slots get combine weight 0). Host un-shards:
every token's output is the sum of its two expert rows.

Perf notes (from perfetto traces of the previous version):
 - ~10us of head latency is framework preamble (6.6us) + first-operand DMA.
   Inputs are host-packed into SBUF-image DRAM layouts so the whole input
   set needs ~15 descriptor issues (vs ~150 at 0.6us each), and group 0's
   x/W1 chunks stream in 256-512KB bundles consumed k-phase by k-phase, so
   the PE never starves more than ~0.5us at the start.
 - HAM clock-gate: the PE runs at 1.2 GHz until ~3.4us of sustained
   activity. Dummy warm-up matmuls on a memset tile run during the
   preamble/DMA window so real matmuls start at 2.4 GHz.
 - mm1 groups must keep moving-dim >= 256: a 128-token group is
   LDWEIGHTS-bound (~81ns/MM for 53ns of streaming). Token groups are
   [512,512,512,384,256] instead of [512x4,128].
"""

import numpy as np
import ml_dtypes

B, T, D, H, E = 4, 2048, 1024, 2048, 8
TOP_K = 2
N = B * T
P = 128
NCORES = 8
MM_FREE = 512  # PSUM bank-limited matmul free dim (fp32 out)
KD = D // P    # contraction tiles for x @ W1
KH = H // P    # contraction tiles for h @ W2
NJ = D // MM_FREE
NW1C = H // MM_FREE  # w1 column chunks

_compiled = {}


def _plan_groups(C):
    """Split C (multiple of 128) into mm1 token groups, each a multiple of
    128 with 256 <= size <= 512 (small moving dims are LDWEIGHTS-bound)."""
    assert C % P == 0 and C >= 512
    groups = []
    rem = C
    while rem > 768:
        groups.append(512)
        rem -= 512
    if rem > 512:  # 640 or 768
        groups.append(rem - 256)
        rem = 256
    groups.append(rem)
    return groups


def _build(C):
    import concourse.bacc as bacc
    import concourse.mybir as mybir
    import concourse.tile as tile

    fp32 = mybir.dt.float32
    bf16 = mybir.dt.bfloat16

    gsizes = _plan_groups(C)
    g0 = gsizes[0]          # first group (512 unless C < 512+256)
    Cr = C - g0             # tokens in the remaining groups
    # token offset of each group
    goff = [0]
    for gs in gsizes:
        goff.append(goff[-1] + gs)

    nc = bacc.Bacc("TRN2", target_bir_lowering=False, debug=False)

    # SBUF-image DRAM layouts (one DMA lands exactly one SBUF tile image):
    #   xg0  [P, KD*g0]   x^T chunks k=0..7 of group 0, k-major
    #   xg12 [P, KD*n12]  x^T chunks of groups 1..2, k-major
    #   xg34 [P, KD*n34]  x^T chunks of groups 3..., k-major
    #   w1c{c} [P, KD*MM_FREE]  W1 column block c, k-major
    #   w2i  [P, KH*D]    W2 row chunks i=0..15
    #   wt   [P, C//P]    combine weights, token t at [t%P, t//P]
    n12 = sum(gsizes[1:3]) if len(gsizes) > 1 else 0
    n34 = Cr - n12
    xg0_d = nc.dram_tensor("xg0", [P, KD * g0], bf16, kind="ExternalInput").ap()
    xg12_d = xg34_d = None
    if n12:
        xg12_d = nc.dram_tensor("xg12", [P, KD * n12], bf16, kind="ExternalInput").ap()
    if n34:
        xg34_d = nc.dram_tensor("xg34", [P, KD * n34], bf16, kind="ExternalInput").ap()
    w1c_d = [nc.dram_tensor(f"w1c{c}", [P, KD * MM_FREE], bf16, kind="ExternalInput").ap()
             for c in range(NW1C)]
    w2_d = nc.dram_tensor("w2i", [P, KH * D], bf16, kind="ExternalInput").ap()
    wt_d = nc.dram_tensor("wt", [P, C // P], fp32, kind="ExternalInput").ap()
    out = nc.dram_tensor("out", [C, D], fp32, kind="ExternalOutput").ap()

    with tile.TileContext(nc) as tc:
        with (
            tc.tile_pool(name="persist", bufs=1) as persist,
            tc.tile_pool(name="hpool", bufs=2 * KH + 2) as hpool,
            tc.tile_pool(name="opool", bufs=4) as opool,
            tc.tile_pool(name="psum1", bufs=4, space="PSUM") as psum1,
            tc.tile_pool(name="psum2", bufs=4, space="PSUM") as psum2,
        ):
            # --- PE warm-up: HAM un-throttles after ~3.4us of activity; run
            # dummy matmuls on a memset tile so real matmuls start at 2.4GHz.
            warm = persist.tile([P, MM_FREE], bf16, tag="warm", name="warm")
            nc.gpsimd.memset(warm[:, :], 0.0)
            for r in range(6):
                wps = psum2.tile([P, MM_FREE], fp32, tag="ps2", name=f"warm_{r}")
                nc.tensor.matmul(wps, warm[:, :P], warm, start=True, stop=True)

            # --- input DMA: everything on the sync queue in strict priority
            # order (single queue => descriptors execute exactly in issue
            # order; the ~512-descriptor ring backpressures later issues
            # harmlessly). Group-0 x/W1 k-pair bundles interleave first so
            # the k-phased first i-sweep consumes them as they land.
            w1_parts = {0: 4, 1: 2}  # bundle counts per w1 column block
            w1_sb = [None] * NW1C    # (bundle tiles, ks-per-bundle)
            xg0_sb = []              # 4 tiles, k-pair bundles of group 0

            def load_w1_part(c, p):
                npart = w1_parts.get(c, 1)
                kper = KD // npart
                w = kper * MM_FREE
                if w1_sb[c] is None:
                    w1_sb[c] = ([None] * npart, kper)
                t = persist.tile([P, w], bf16, tag=f"w1_{c}_{p}",
                                 name=f"w1_{c}_{p}")
                nc.sync.dma_start(out=t, in_=w1c_d[c][:, p * w:(p + 1) * w])
                w1_sb[c][0][p] = t

            for p in range(KD // 2):
                load_w1_part(0, p)
                t = persist.tile([P, 2 * g0], bf16, tag=f"xg0_{p}", name=f"xg0_{p}")
                nc.sync.dma_start(out=t, in_=xg0_d[:, 2 * p * g0:(2 * p + 2) * g0])
                xg0_sb.append(t)
            for c in range(1, NW1C):
                for p in range(w1_parts.get(c, 1)):
                    load_w1_part(c, p)
            xg12_sb = xg34_sb = None
            if n12:
                xg12_sb = persist.tile([P, KD * n12], bf16, tag="xg12", name="xg12")
                nc.sync.dma_start(out=xg12_sb, in_=xg12_d[:, :])
            if n34:
                xg34_sb = persist.tile([P, KD * n34], bf16, tag="xg34", name="xg34")
                nc.sync.dma_start(out=xg34_sb, in_=xg34_d[:, :])
            w2_sb = []
            for p in range(2):
                w = KH * D // 2
                t = persist.tile([P, w], bf16, tag=f"w2_{p}", name=f"w2_{p}")
                nc.sync.dma_start(out=t, in_=w2_d[:, p * w:(p + 1) * w])
                w2_sb.append(t)
            wt_sb = persist.tile([P, C // P], fp32, tag="wt", name="wt_sb")
            nc.sync.dma_start(out=wt_sb, in_=wt_d[:, :])

            def xT_view(k, gi):
                """[P, gsizes[gi]] view of x^T chunk k for group gi."""
                t0, gs = goff[gi], gsizes[gi]
                if gi == 0:
                    return xg0_sb[k // 2][:, (k % 2) * g0:(k % 2) * g0 + g0]
                if gi <= 2 and n12:
                    o = t0 - g0
                    return xg12_sb[:, k * n12 + o:k * n12 + o + gs]
                o = t0 - g0 - n12
                return xg34_sb[:, k * n34 + o:k * n34 + o + gs]

            def w1_view(k, c, co):
                """[P, P] stationary: W1 rows k*128.., cols c*512+co."""
                tiles, kper = w1_sb[c]
                o = (k % kper) * MM_FREE + co
                return tiles[k // kper][:, o:o + P]

            def w2_view(i, j):
                """[P, MM_FREE]: W2 rows i*128.., cols j*512.."""
                kper = KH // 2
                o = (i % kper) * D + j * MM_FREE
                return w2_sb[i // kper][:, o:o + MM_FREE]

            # --- software-pipelined groups: mm1(g0), mm1(g1), mm2(g0), ...
            def mm1(gi):
                gs = gsizes[gi]
                hts = []
                if gi == 0:
                    # k-phased first i-sweep: i=0..3 accumulate k-pairs as the
                    # xg0/w1c0 bundles land (start only needs 512KB in SBUF).
                    pss = [psum1.tile([P, MM_FREE], fp32, tag="ps1", name=f"ps1_0_{i}")
                           for i in range(4)]
                    for kp in range(KD // 2):
                        for i in range(4):
                            for k in (2 * kp, 2 * kp + 1):
                                nc.tensor.matmul(
                                    pss[i][:, :gs], w1_view(k, 0, i * P),
                                    xT_view(k, 0),
                                    start=(k == 0), stop=(k == KD - 1),
                                )
                    for i in range(4):
                        ht = hpool.tile([P, MM_FREE], bf16, tag="hT", name=f"hT_0_{i}")
                        nc.scalar.activation(
                            ht[:, :gs], pss[i][:, :gs],
                            mybir.ActivationFunctionType.Silu,
                        )
                        hts.append(ht)
                    irange = range(4, KH)
                else:
                    irange = range(KH)
                for i in irange:
                    ci, co = divmod(i * P, MM_FREE)
                    ps = psum1.tile([P, MM_FREE], fp32, tag="ps1", name=f"ps1_{gi}_{i}")
                    for k in range(KD):
                        nc.tensor.matmul(
                            ps[:, :gs], w1_view(k, ci, co), xT_view(k, gi),
                            start=(k == 0), stop=(k == KD - 1),
                        )
                    ht = hpool.tile([P, MM_FREE], bf16, tag="hT", name=f"hT_{gi}_{i}")
                    nc.scalar.activation(
                        ht[:, :gs], ps[:, :gs], mybir.ActivationFunctionType.Silu
                    )
                    hts.append(ht)
                return hts

            def mm2(gi, hts):
                t0, gs = goff[gi], gsizes[gi]
                for t in range(gs // P):
                    tok = t0 + t * P
                    for j in range(NJ):
                        ps2 = psum2.tile([P, MM_FREE], fp32, tag="ps2",
                                         name=f"ps2_{tok}_{j}")
                        for i in range(KH):
                            nc.tensor.matmul(
                                ps2,
                                hts[i][:, t * P:(t + 1) * P],
                                w2_view(i, j),
                                start=(i == 0), stop=(i == KH - 1),
                            )
                        ot = opool.tile([P, MM_FREE], fp32, tag="ot",
                                        name=f"ot_{tok}_{j}")
                        nc.vector.tensor_scalar_mul(
                            ot, ps2, wt_sb[:, tok // P:tok // P + 1]
                        )
                        nc.sync.dma_start(
                            out=out[tok:tok + P, j * MM_FREE:(j + 1) * MM_FREE], in_=ot
                        )

            prev = (0, mm1(0))
            for gi in range(1, len(gsizes)):
                hts = mm1(gi)
                mm2(*prev)
                prev = (gi, hts)
            mm2(*prev)

    nc.compile()
    return nc


def _get_compiled(C):
    if C not in _compiled:
        _compiled[C] = _build(C)
    return _compiled[C]


def _pack_xT(xTe, g0, n12, n34):
    """Split x^T [D, C] into the k-major SBUF-image layouts."""
    arr = np.ascontiguousarray(xTe).reshape(KD, P, xTe.shape[1])
    m = {"xg0": np.ascontiguousarray(
        arr[:, :, :g0].transpose(1, 0, 2).reshape(P, KD * g0))}
    if n12:
        m["xg12"] = np.ascontiguousarray(
            arr[:, :, g0:g0 + n12].transpose(1, 0, 2).reshape(P, KD * n12))
    if n34:
        m["xg34"] = np.ascontiguousarray(
            arr[:, :, g0 + n12:].transpose(1, 0, 2).reshape(P, KD * n34))
    return m


def kernel(**inputs):
    x = np.asarray(inputs["x"], dtype=np.float32)
    Wg = np.asarray(inputs["Wg"], dtype=np.float32)
    W1 = np.asarray(inputs["W1"], dtype=np.float32)
    W2 = np.asarray(inputs["W2"], dtype=np.float32)
    xf = np.ascontiguousarray(x.reshape(-1, D))

    # --- host-side gate + top-2 routing (float64; ordering matches f32 ref) ---
    logits = xf.astype(np.float64) @ Wg.astype(np.float64)
    w = np.exp(logits - logits.max(axis=-1, keepdims=True))
    w /= w.sum(axis=-1, keepdims=True)
    order = np.argsort(-w, axis=-1, kind="stable")[:, :TOP_K]  # [N, 2] expert ids
    tw = np.take_along_axis(w, order, axis=-1)
    tw = tw / tw.sum(axis=-1, keepdims=True)  # renormalized combine weights

    counts = np.bincount(order.ravel(), minlength=E)
    C = int(-(-max(int(counts.max()), 512) // P) * P)
    # per-partition SBUF: xT images 16*C bytes + ~107KB of weights/pools
    assert 16 * C + 110 * 1024 < 200 * 1024, "pathological routing skew"

    nc = _get_compiled(C)
    gsizes = _plan_groups(C)
    g0 = gsizes[0]
    n12 = sum(gsizes[1:3]) if len(gsizes) > 1 else 0
    n34 = C - g0 - n12

    bf = ml_dtypes.bfloat16
    pos = np.empty((N, TOP_K), dtype=np.int64)
    in_maps = []
    for e in range(E):
        sel = np.nonzero((order == e).any(axis=-1))[0]
        slot = (order[sel, 1] == e).astype(np.int64)
        pos[sel, slot] = e * C + np.arange(len(sel))

        xTe = np.zeros((D, C), dtype=bf)
        xTe[:, :len(sel)] = xf[sel].T.astype(bf)
        wtp = np.zeros(C, dtype=np.float32)
        wtp[:len(sel)] = tw[sel, slot].astype(np.float32)

        m = _pack_xT(xTe, g0, n12, n34)
        W1e = np.ascontiguousarray(W1[e]).astype(bf).reshape(KD, P, NW1C, MM_FREE)
        for c in range(NW1C):
            m[f"w1c{c}"] = np.ascontiguousarray(
                W1e[:, :, c, :].transpose(1, 0, 2).reshape(P, KD * MM_FREE))
        m["w2i"] = np.ascontiguousarray(
            np.ascontiguousarray(W2[e]).astype(bf).reshape(KH, P, D)
            .transpose(1, 0, 2).reshape(P, KH * D))
        m["wt"] = np.ascontiguousarray(wtp.reshape(C // P, P).T)
        in_maps.append(m)

    from concourse.bass_utils import run_bass_kernel_spmd

    # The SPMD launch reaches the 8 NeuronCores through jax/PJRT. If the
    # calling process pinned jax to CPU (e.g. to run the reference), flip to
    # the axon platform for the launch and restore afterwards.
    import jax

    flipped = False
    try:
        n_acc = len([d for d in jax.devices() if d.platform != "cpu"])
    except Exception:
        n_acc = 0

    def _clear_backends():
        try:
            import jax.extend.backend as jeb
            jeb.clear_backends()
        except Exception:
            from jax._src import xla_bridge
            xla_bridge._clear_backends()

    if n_acc < NCORES:
        prev = jax.config.jax_platforms
        jax.config.update("jax_platforms", "axon")
        _clear_backends()
        flipped = True
    try:
        res = run_bass_kernel_spmd(nc, in_maps, core_ids=list(range(NCORES)))
    finally:
        if flipped:
            jax.config.update("jax_platforms", prev)
            _clear_backends()

    Y = np.concatenate([res.results[c]["out"] for c in range(NCORES)], axis=0)
    outf = Y[pos[:, 0]] + Y[pos[:, 1]]
    return outf.reshape(B, T, D).astype(np.float32)


# revision 15
# speedup vs baseline: 1.0092x; 1.0018x over previous
"""MoE feed-forward (top-2 of 8 experts) on 8 Trainium2 NeuronCores.

Strategy: expert-parallel, primary-only. Core e owns expert e's weights and
processes every token routed to expert e (capacity C = ceil128(max expert
count); for the graded distribution max count 2151 -> C = 2176, the same
device cost as the old primary+secondary scheme but with half the weight
DMA and a simpler dispatch). The (tiny) gate runs on host in float64
(ordering robust: min rank-2/rank-3 weight gap ~6.6e-6 >> f32 noise).

Each core computes, in bf16 with f32 PSUM accumulation,

    out_row = (silu(x_row @ W1[e]) @ W2[e]) * combine_weight_row

for its tokens (zero-padded slots get combine weight 0). Host un-shards:
every token's output is the sum of its two expert rows.

Perf notes (from perfetto traces of the previous version):
 - ~10us of head latency is framework preamble (6.6us) + first-operand DMA.
   Inputs are host-packed into SBUF-image DRAM layouts so the whole input
   set needs ~15 descriptor issues (vs ~150 at 0.6us each), and group 0's
   x/W1 chunks stream in 256-512KB bundles consumed k-phase by k-phase, so
   the PE never starves more than ~0.5us at the start.
 - HAM clock-gate: the PE runs at 1.2 GHz until ~3.4us of sustained
   activity. Dummy warm-up matmuls on a memset tile run during the
   preamble/DMA window so real matmuls start at 2.4 GHz.
 - mm1 groups must keep moving-dim >= 256: a 128-token group is
   LDWEIGHTS-bound (~81ns/MM for 53ns of streaming). Token groups are
   [512,512,512,384,256] instead of [512x4,128].
"""

import numpy as np
import ml_dtypes

B, T, D, H, E = 4, 2048, 1024, 2048, 8
TOP_K = 2
N = B * T
P = 128
NCORES = 8
MM_FREE = 512  # PSUM bank-limited matmul free dim (fp32 out)
KD = D // P    # contraction tiles for x @ W1
KH = H // P    # contraction tiles for h @ W2
NJ = D // MM_FREE
NW1C = H // MM_FREE  # w1 column chunks

_compiled = {}


def _plan_groups(C):
    """Split C (multiple of 128) into mm1 token groups, each a multiple of
    128 with 256 <= size <= 512 (small moving dims are LDWEIGHTS-bound)."""
    assert C % P == 0 and C >= 512
    groups = []
    rem = C
    while rem > 768:
        groups.append(512)
        rem -= 512
    if rem > 512:  # 640 or 768
        groups.append(rem - 256)
        rem = 256
    groups.append(rem)
    return groups


def _build(C):
    import concourse.bacc as bacc
    import concourse.mybir as mybir
    import concourse.tile as tile

    fp32 = mybir.dt.float32
    bf16 = mybir.dt.bfloat16

    gsizes = _plan_groups(C)
    g0 = gsizes[0]          # first group (512 unless C < 512+256)
    Cr = C - g0             # tokens in the remaining groups
    # token offset of each group
    goff = [0]
    for gs in gsizes:
        goff.append(goff[-1] + gs)

    nc = bacc.Bacc("TRN2", target_bir_lowering=False, debug=False)

    # SBUF-image DRAM layouts (one DMA lands exactly one SBUF tile image):
    #   xg0  [P, KD*g0]   x^T chunks k=0..7 of group 0, k-major
    #   xg12 [P, KD*n12]  x^T chunks of groups 1..2, k-major
    #   xg34 [P, KD*n34]  x^T chunks of groups 3..., k-major
    #   w1c{c} [P, KD*MM_FREE]  W1 column block c, k-major
    #   w2i  [P, KH*D]    W2 row chunks i=0..15
    #   wt   [P, C//P]    combine weights, token t at [t%P, t//P]
    n12 = sum(gsizes[1:3]) if len(gsizes) > 1 else 0
    n34 = Cr - n12
    xg0_d = nc.dram_tensor("xg0", [P, KD * g0], bf16, kind="ExternalInput").ap()
    xg12_d = xg34_d = None
    if n12:
        xg12_d = nc.dram_tensor("xg12", [P, KD * n12], bf16, kind="ExternalInput").ap()
    if n34:
        xg34_d = nc.dram_tensor("xg34", [P, KD * n34], bf16, kind="ExternalInput").ap()
    w1c_d = [nc.dram_tensor(f"w1c{c}", [P, KD * MM_FREE], bf16, kind="ExternalInput").ap()
             for c in range(NW1C)]
    w2_d = nc.dram_tensor("w2i", [P, KH * D], bf16, kind="ExternalInput").ap()
    wt_d = nc.dram_tensor("wt", [P, C // P], fp32, kind="ExternalInput").ap()
    out = nc.dram_tensor("out", [C, D], fp32, kind="ExternalOutput").ap()

    with tile.TileContext(nc) as tc:
        with (
            tc.tile_pool(name="persist", bufs=1) as persist,
            tc.tile_pool(name="hpool", bufs=2 * KH + 2) as hpool,
            tc.tile_pool(name="opool", bufs=4) as opool,
            tc.tile_pool(name="psum1", bufs=4, space="PSUM") as psum1,
            tc.tile_pool(name="psum2", bufs=4, space="PSUM") as psum2,
        ):
            # --- PE warm-up: HAM un-throttles after ~3.4us of activity; run
            # dummy matmuls on a memset tile so real matmuls start at 2.4GHz.
            warm = persist.tile([P, MM_FREE], bf16, tag="warm", name="warm")
            nc.gpsimd.memset(warm[:, :], 0.0)
            for r in range(5):
                wps = psum2.tile([P, MM_FREE], fp32, tag="ps2", name=f"warm_{r}")
                nc.tensor.matmul(wps, warm[:, :P], warm, start=True, stop=True)

            # --- input DMA: everything on the sync queue in strict priority
            # order (single queue => descriptors execute exactly in issue
            # order; the ~512-descriptor ring backpressures later issues
            # harmlessly). Group-0 x/W1 bundles interleave first, finest
            # (128KB) bundles leading, so the k-phased first i-sweep starts
            # ~1.4us after the first descriptor and never starves.
            KPHASES = [(0,), (1,), (2, 3), (4, 5), (6, 7)]
            w1_parts = {0: KPHASES, 1: KPHASES}
            w1_sb = [None] * NW1C    # (bundle tiles, k -> (part, off))
            xg0_sb = ([None] * len(KPHASES), {})  # tiles, k -> (part, off)

            def load_w1_part(c, p):
                phases = w1_parts.get(c, [tuple(range(KD))])
                ks = phases[p]
                if w1_sb[c] is None:
                    kmap = {}
                    for pp, pks in enumerate(phases):
                        for o, k in enumerate(pks):
                            kmap[k] = (pp, o)
                    w1_sb[c] = ([None] * len(phases), kmap)
                t = persist.tile([P, len(ks) * MM_FREE], bf16,
                                 tag=f"w1_{c}_{p}", name=f"w1_{c}_{p}")
                nc.sync.dma_start(
                    out=t, in_=w1c_d[c][:, ks[0] * MM_FREE:(ks[-1] + 1) * MM_FREE])
                w1_sb[c][0][p] = t

            for p, ks in enumerate(KPHASES):
                load_w1_part(0, p)
                load_w1_part(1, p)
                t = persist.tile([P, len(ks) * g0], bf16,
                                 tag=f"xg0_{p}", name=f"xg0_{p}")
                nc.sync.dma_start(
                    out=t, in_=xg0_d[:, ks[0] * g0:(ks[-1] + 1) * g0])
                xg0_sb[0][p] = t
                for o, k in enumerate(ks):
                    xg0_sb[1][k] = (p, o)
            for c in range(2, NW1C):
                for p in range(len(w1_parts.get(c, [0]))):
                    load_w1_part(c, p)
            xg12_sb = xg34_sb = None
            if n12:
                xg12_sb = persist.tile([P, KD * n12], bf16, tag="xg12", name="xg12")
                nc.sync.dma_start(out=xg12_sb, in_=xg12_d[:, :])
            if n34:
                xg34_sb = persist.tile([P, KD * n34], bf16, tag="xg34", name="xg34")
                nc.sync.dma_start(out=xg34_sb, in_=xg34_d[:, :])
            w2_sb = []
            for p in range(2):
                w = KH * D // 2
                t = persist.tile([P, w], bf16, tag=f"w2_{p}", name=f"w2_{p}")
                nc.sync.dma_start(out=t, in_=w2_d[:, p * w:(p + 1) * w])
                w2_sb.append(t)
            wt_sb = persist.tile([P, C // P], fp32, tag="wt", name="wt_sb")
            nc.sync.dma_start(out=wt_sb, in_=wt_d[:, :])

            def xT_view(k, gi):
                """[P, gsizes[gi]] view of x^T chunk k for group gi."""
                t0, gs = goff[gi], gsizes[gi]
                if gi == 0:
                    p, o = xg0_sb[1][k]
                    return xg0_sb[0][p][:, o * g0:o * g0 + g0]
                if gi <= 2 and n12:
                    o = t0 - g0
                    return xg12_sb[:, k * n12 + o:k * n12 + o + gs]
                o = t0 - g0 - n12
                return xg34_sb[:, k * n34 + o:k * n34 + o + gs]

            def w1_view(k, c, co):
                """[P, P] stationary: W1 rows k*128.., cols c*512+co."""
                tiles, kmap = w1_sb[c]
                p, o = kmap[k]
                return tiles[p][:, o * MM_FREE + co:o * MM_FREE + co + P]

            def w2_view(i, j):
                """[P, MM_FREE]: W2 rows i*128.., cols j*512.."""
                kper = KH // 2
                o = (i % kper) * D + j * MM_FREE
                return w2_sb[i // kper][:, o:o + MM_FREE]

            # --- software-pipelined groups: mm1(g0), mm1(g1), mm2(g0), ...
            def mm1(gi):
                gs = gsizes[gi]
                hts = []
                if gi == 0:
                    # k-phased first i-sweep: i=0..7 accumulate into all 8
                    # PSUM banks as the xg0/w1c0 bundles land (the first MM
                    # needs only 256KB in SBUF; delivery outruns compute).
                    pss = [(psum1 if i < 4 else psum2).tile(
                               [P, MM_FREE], fp32, tag="ps1" if i < 4 else "ps2",
                               name=f"ps1_0_{i}")
                           for i in range(8)]
                    for ks in KPHASES:
                        for i in range(8):
                            ci, co = divmod(i * P, MM_FREE)
                            for k in ks:
                                nc.tensor.matmul(
                                    pss[i][:, :gs], w1_view(k, ci, co),
                                    xT_view(k, 0),
                                    start=(k == 0), stop=(k == KD - 1),
                                )
                    for i in range(8):
                        ht = hpool.tile([P, MM_FREE], bf16, tag="hT", name=f"hT_0_{i}")
                        nc.scalar.activation(
                            ht[:, :gs], pss[i][:, :gs],
                            mybir.ActivationFunctionType.Silu,
                        )
                        hts.append(ht)
                    irange = range(8, KH)
                else:
                    irange = range(KH)
                for i in irange:
                    ci, co = divmod(i * P, MM_FREE)
                    ps = psum1.tile([P, MM_FREE], fp32, tag="ps1", name=f"ps1_{gi}_{i}")
                    for k in range(KD):
                        nc.tensor.matmul(
                            ps[:, :gs], w1_view(k, ci, co), xT_view(k, gi),
                            start=(k == 0), stop=(k == KD - 1),
                        )
                    ht = hpool.tile([P, MM_FREE], bf16, tag="hT", name=f"hT_{gi}_{i}")
                    nc.scalar.activation(
                        ht[:, :gs], ps[:, :gs], mybir.ActivationFunctionType.Silu
                    )
                    hts.append(ht)
                return hts

            def mm2(gi, hts):
                t0, gs = goff[gi], gsizes[gi]
                for t in range(gs // P):
                    tok = t0 + t * P
                    for j in range(NJ):
                        ps2 = psum2.tile([P, MM_FREE], fp32, tag="ps2",
                                         name=f"ps2_{tok}_{j}")
                        for i in range(KH):
                            nc.tensor.matmul(
                                ps2,
                                hts[i][:, t * P:(t + 1) * P],
                                w2_view(i, j),
                                start=(i == 0), stop=(i == KH - 1),
                            )
                        ot = opool.tile([P, MM_FREE], fp32, tag="ot",
                                        name=f"ot_{tok}_{j}")
                        nc.vector.tensor_scalar_mul(
                            ot, ps2, wt_sb[:, tok // P:tok // P + 1]
                        )
                        nc.sync.dma_start(
                            out=out[tok:tok + P, j * MM_FREE:(j + 1) * MM_FREE], in_=ot
                        )

            prev = (0, mm1(0))
            for gi in range(1, len(gsizes)):
                hts = mm1(gi)
                mm2(*prev)
                prev = (gi, hts)
            mm2(*prev)

    nc.compile()
    return nc


def _get_compiled(C):
    if C not in _compiled:
        _compiled[C] = _build(C)
    return _compiled[C]


def _pack_xT(xTe, g0, n12, n34):
    """Split x^T [D, C] into the k-major SBUF-image layouts."""
    arr = np.ascontiguousarray(xTe).reshape(KD, P, xTe.shape[1])
    m = {"xg0": np.ascontiguousarray(
        arr[:, :, :g0].transpose(1, 0, 2).reshape(P, KD * g0))}
    if n12:
        m["xg12"] = np.ascontiguousarray(
            arr[:, :, g0:g0 + n12].transpose(1, 0, 2).reshape(P, KD * n12))
    if n34:
        m["xg34"] = np.ascontiguousarray(
            arr[:, :, g0 + n12:].transpose(1, 0, 2).reshape(P, KD * n34))
    return m


def kernel(**inputs):
    x = np.asarray(inputs["x"], dtype=np.float32)
    Wg = np.asarray(inputs["Wg"], dtype=np.float32)
    W1 = np.asarray(inputs["W1"], dtype=np.float32)
    W2 = np.asarray(inputs["W2"], dtype=np.float32)
    xf = np.ascontiguousarray(x.reshape(-1, D))

    # --- host-side gate + top-2 routing (float64; ordering matches f32 ref) ---
    logits = xf.astype(np.float64) @ Wg.astype(np.float64)
    w = np.exp(logits - logits.max(axis=-1, keepdims=True))
    w /= w.sum(axis=-1, keepdims=True)
    order = np.argsort(-w, axis=-1, kind="stable")[:, :TOP_K]  # [N, 2] expert ids
    tw = np.take_along_axis(w, order, axis=-1)
    tw = tw / tw.sum(axis=-1, keepdims=True)  # renormalized combine weights

    counts = np.bincount(order.ravel(), minlength=E)
    C = int(-(-max(int(counts.max()), 512) // P) * P)
    # per-partition SBUF: xT images 16*C bytes + ~107KB of weights/pools
    assert 16 * C + 110 * 1024 < 200 * 1024, "pathological routing skew"

    nc = _get_compiled(C)
    gsizes = _plan_groups(C)
    g0 = gsizes[0]
    n12 = sum(gsizes[1:3]) if len(gsizes) > 1 else 0
    n34 = C - g0 - n12

    bf = ml_dtypes.bfloat16
    pos = np.empty((N, TOP_K), dtype=np.int64)
    in_maps = []
    for e in range(E):
        sel = np.nonzero((order == e).any(axis=-1))[0]
        slot = (order[sel, 1] == e).astype(np.int64)
        pos[sel, slot] = e * C + np.arange(len(sel))

        xTe = np.zeros((D, C), dtype=bf)
        xTe[:, :len(sel)] = xf[sel].T.astype(bf)
        wtp = np.zeros(C, dtype=np.float32)
        wtp[:len(sel)] = tw[sel, slot].astype(np.float32)

        m = _pack_xT(xTe, g0, n12, n34)
        W1e = np.ascontiguousarray(W1[e]).astype(bf).reshape(KD, P, NW1C, MM_FREE)
        for c in range(NW1C):
            m[f"w1c{c}"] = np.ascontiguousarray(
                W1e[:, :, c, :].transpose(1, 0, 2).reshape(P, KD * MM_FREE))
        m["w2i"] = np.ascontiguousarray(
            np.ascontiguousarray(W2[e]).astype(bf).reshape(KH, P, D)
            .transpose(1, 0, 2).reshape(P, KH * D))
        m["wt"] = np.ascontiguousarray(wtp.reshape(C // P, P).T)
        in_maps.append(m)

    from concourse.bass_utils import run_bass_kernel_spmd

    # The SPMD launch reaches the 8 NeuronCores through jax/PJRT. If the
    # calling process pinned jax to CPU (e.g. to run the reference), flip to
    # the axon platform for the launch and restore afterwards.
    import jax

    flipped = False
    try:
        n_acc = len([d for d in jax.devices() if d.platform != "cpu"])
    except Exception:
        n_acc = 0

    def _clear_backends():
        try:
            import jax.extend.backend as jeb
            jeb.clear_backends()
        except Exception:
            from jax._src import xla_bridge
            xla_bridge._clear_backends()

    if n_acc < NCORES:
        prev = jax.config.jax_platforms
        jax.config.update("jax_platforms", "axon")
        _clear_backends()
        flipped = True
    try:
        res = run_bass_kernel_spmd(nc, in_maps, core_ids=list(range(NCORES)))
    finally:
        if flipped:
            jax.config.update("jax_platforms", prev)
            _clear_backends()

    Y = np.concatenate([res.results[c]["out"] for c in range(NCORES)], axis=0)
    outf = Y[pos[:, 0]] + Y[pos[:, 1]]
    return outf.reshape(B, T, D).astype(np.float32)


# revision 17
# speedup vs baseline: 1.0161x; 1.0068x over previous
"""MoE feed-forward (top-2 of 8 experts) on 8 Trainium2 NeuronCores.

Strategy: expert-parallel, primary-only. Core e owns expert e's weights and
processes every token routed to expert e (capacity C = ceil128(max expert
count); for the graded distribution max count 2151 -> C = 2176, the same
device cost as the old primary+secondary scheme but with half the weight
DMA and a simpler dispatch). The (tiny) gate runs on host in float64
(ordering robust: min rank-2/rank-3 weight gap ~6.6e-6 >> f32 noise).

Each core computes, in bf16 with f32 PSUM accumulation,

    out_row = (silu(x_row @ W1[e]) @ W2[e]) * combine_weight_row

for its tokens (zero-padded slots get combine weight 0). Host un-shards:
every token's output is the sum of its two expert rows.

Perf notes (from perfetto traces of the previous version):
 - ~10us of head latency is framework preamble (6.6us) + first-operand DMA.
   Inputs are host-packed into SBUF-image DRAM layouts so the whole input
   set needs ~15 descriptor issues (vs ~150 at 0.6us each), and group 0's
   x/W1 chunks stream in 256-512KB bundles consumed k-phase by k-phase, so
   the PE never starves more than ~0.5us at the start.
 - HAM clock-gate: the PE runs at 1.2 GHz until ~3.4us of sustained
   activity. Dummy warm-up matmuls on a memset tile run during the
   preamble/DMA window so real matmuls start at 2.4 GHz.
 - mm1 groups must keep moving-dim >= 256: a 128-token group is
   LDWEIGHTS-bound (~81ns/MM for 53ns of streaming). Token groups are
   [512,512,512,384,256] instead of [512x4,128].
"""

import numpy as np
import ml_dtypes

B, T, D, H, E = 4, 2048, 1024, 2048, 8
TOP_K = 2
N = B * T
P = 128
NCORES = 8
MM_FREE = 512  # PSUM bank-limited matmul free dim (fp32 out)
KD = D // P    # contraction tiles for x @ W1
KH = H // P    # contraction tiles for h @ W2
NJ = D // MM_FREE
NW1C = H // MM_FREE  # w1 column chunks

_compiled = {}


def _plan_groups(C):
    """Split C (multiple of 128) into mm1 token groups, each a multiple of
    128 with 256 <= size <= 512 (small moving dims are LDWEIGHTS-bound)."""
    assert C % P == 0 and C >= 512
    groups = []
    rem = C
    while rem > 768:
        groups.append(512)
        rem -= 512
    if rem > 512:  # 640 or 768
        groups.append(rem - 256)
        rem = 256
    groups.append(rem)
    return groups


def _build(C):
    import concourse.bacc as bacc
    import concourse.mybir as mybir
    import concourse.tile as tile

    fp32 = mybir.dt.float32
    bf16 = mybir.dt.bfloat16

    gsizes = _plan_groups(C)
    g0 = gsizes[0]          # first group (512 unless C < 512+256)
    Cr = C - g0             # tokens in the remaining groups
    # token offset of each group
    goff = [0]
    for gs in gsizes:
        goff.append(goff[-1] + gs)

    nc = bacc.Bacc("TRN2", target_bir_lowering=False, debug=False)

    # SBUF-image DRAM layouts (one DMA lands exactly one SBUF tile image):
    #   xg0  [P, KD*g0]   x^T chunks k=0..7 of group 0, k-major
    #   xg12 [P, KD*n12]  x^T chunks of groups 1..2, k-major
    #   xg34 [P, KD*n34]  x^T chunks of groups 3..., k-major
    #   w1c{c} [P, KD*MM_FREE]  W1 column block c, k-major
    #   w2i  [P, KH*D]    W2 row chunks i=0..15
    #   wt   [P, C//P]    combine weights, token t at [t%P, t//P]
    n12 = sum(gsizes[1:3]) if len(gsizes) > 1 else 0
    n34 = Cr - n12
    xg0_d = nc.dram_tensor("xg0", [P, KD * g0], bf16, kind="ExternalInput").ap()
    xg12_d = xg34_d = None
    if n12:
        xg12_d = nc.dram_tensor("xg12", [P, KD * n12], bf16, kind="ExternalInput").ap()
    if n34:
        xg34_d = nc.dram_tensor("xg34", [P, KD * n34], bf16, kind="ExternalInput").ap()
    w1c_d = [nc.dram_tensor(f"w1c{c}", [P, KD * MM_FREE], bf16, kind="ExternalInput").ap()
             for c in range(NW1C)]
    w2_d = nc.dram_tensor("w2i", [P, KH * D], bf16, kind="ExternalInput").ap()
    wt_d = nc.dram_tensor("wt", [P, C // P], fp32, kind="ExternalInput").ap()
    out = nc.dram_tensor("out", [C, D], fp32, kind="ExternalOutput").ap()

    with tile.TileContext(nc) as tc:
        with (
            tc.tile_pool(name="persist", bufs=1) as persist,
            tc.tile_pool(name="hpool", bufs=2 * KH + 2) as hpool,
            tc.tile_pool(name="opool", bufs=4) as opool,
            tc.tile_pool(name="psum1", bufs=4, space="PSUM") as psum1,
            tc.tile_pool(name="psum2", bufs=4, space="PSUM") as psum2,
        ):
            # --- PE warm-up: HAM un-throttles after ~3.4us of activity; run
            # dummy matmuls on a memset tile so real matmuls start at 2.4GHz.
            warm = persist.tile([P, MM_FREE], bf16, tag="warm", name="warm")
            nc.gpsimd.memset(warm[:, :], 0.0)
            # 13 dummy matmuls bridge from ~7.8us (engines ready) to ~13.3us
            # (first operands' DMA-complete semaphore) so HAM latches warm
            # (needs ~3.4us sustained) and real matmuls start at 2.4GHz.
            for r in range(13):
                wps = psum2.tile([P, MM_FREE], fp32, tag="ps2", name=f"warm_{r}")
                nc.tensor.matmul(wps, warm[:, :P], warm, start=True, stop=True)

            # --- input DMA: everything on the sync queue in strict priority
            # order (single queue => descriptors execute exactly in issue
            # order; the ~512-descriptor ring backpressures later issues
            # harmlessly). Group-0 x/W1 bundles interleave first, finest
            # (128KB) bundles leading, so the k-phased first i-sweep starts
            # ~1.4us after the first descriptor and never starves.
            KPHASES = [(0,), (1,), (2, 3), (4, 5), (6, 7)]
            w1_parts = {0: KPHASES, 1: KPHASES}
            w1_sb = [None] * NW1C    # (bundle tiles, k -> (part, off))
            xg0_sb = ([None] * len(KPHASES), {})  # tiles, k -> (part, off)

            def load_w1_part(c, p):
                phases = w1_parts.get(c, [tuple(range(KD))])
                ks = phases[p]
                if w1_sb[c] is None:
                    kmap = {}
                    for pp, pks in enumerate(phases):
                        for o, k in enumerate(pks):
                            kmap[k] = (pp, o)
                    w1_sb[c] = ([None] * len(phases), kmap)
                t = persist.tile([P, len(ks) * MM_FREE], bf16,
                                 tag=f"w1_{c}_{p}", name=f"w1_{c}_{p}")
                nc.sync.dma_start(
                    out=t, in_=w1c_d[c][:, ks[0] * MM_FREE:(ks[-1] + 1) * MM_FREE])
                w1_sb[c][0][p] = t

            for p, ks in enumerate(KPHASES):
                load_w1_part(0, p)
                t = persist.tile([P, len(ks) * g0], bf16,
                                 tag=f"xg0_{p}", name=f"xg0_{p}")
                nc.sync.dma_start(
                    out=t, in_=xg0_d[:, ks[0] * g0:(ks[-1] + 1) * g0])
                xg0_sb[0][p] = t
                for o, k in enumerate(ks):
                    xg0_sb[1][k] = (p, o)
                load_w1_part(1, p)
            for c in range(2, NW1C):
                for p in range(len(w1_parts.get(c, [0]))):
                    load_w1_part(c, p)
            xg12_sb = xg34_sb = None
            if n12:
                xg12_sb = persist.tile([P, KD * n12], bf16, tag="xg12", name="xg12")
                nc.sync.dma_start(out=xg12_sb, in_=xg12_d[:, :])
            if n34:
                xg34_sb = persist.tile([P, KD * n34], bf16, tag="xg34", name="xg34")
                nc.sync.dma_start(out=xg34_sb, in_=xg34_d[:, :])
            w2_sb = []
            for p in range(2):
                w = KH * D // 2
                t = persist.tile([P, w], bf16, tag=f"w2_{p}", name=f"w2_{p}")
                nc.sync.dma_start(out=t, in_=w2_d[:, p * w:(p + 1) * w])
                w2_sb.append(t)
            wt_sb = persist.tile([P, C // P], fp32, tag="wt", name="wt_sb")
            nc.sync.dma_start(out=wt_sb, in_=wt_d[:, :])

            def xT_view(k, gi):
                """[P, gsizes[gi]] view of x^T chunk k for group gi."""
                t0, gs = goff[gi], gsizes[gi]
                if gi == 0:
                    p, o = xg0_sb[1][k]
                    return xg0_sb[0][p][:, o * g0:o * g0 + g0]
                if gi <= 2 and n12:
                    o = t0 - g0
                    return xg12_sb[:, k * n12 + o:k * n12 + o + gs]
                o = t0 - g0 - n12
                return xg34_sb[:, k * n34 + o:k * n34 + o + gs]

            def w1_view(k, c, co):
                """[P, P] stationary: W1 rows k*128.., cols c*512+co."""
                tiles, kmap = w1_sb[c]
                p, o = kmap[k]
                return tiles[p][:, o * MM_FREE + co:o * MM_FREE + co + P]

            def w2_view(i, j):
                """[P, MM_FREE]: W2 rows i*128.., cols j*512.."""
                kper = KH // 2
                o = (i % kper) * D + j * MM_FREE
                return w2_sb[i // kper][:, o:o + MM_FREE]

            # --- software-pipelined groups: mm1(g0), mm1(g1), mm2(g0), ...
            def mm1(gi):
                gs = gsizes[gi]
                hts = []
                if gi == 0:
                    # k-phased first i-sweep: i=0..7 accumulate into all 8
                    # PSUM banks as the xg0/w1c0 bundles land (the first MM
                    # needs only 256KB in SBUF; delivery outruns compute).
                    pss = [(psum1 if i < 4 else psum2).tile(
                               [P, MM_FREE], fp32, tag="ps1" if i < 4 else "ps2",
                               name=f"ps1_0_{i}")
                           for i in range(8)]
                    for ks in KPHASES:
                        for i in range(8):
                            ci, co = divmod(i * P, MM_FREE)
                            for k in ks:
                                nc.tensor.matmul(
                                    pss[i][:, :gs], w1_view(k, ci, co),
                                    xT_view(k, 0),
                                    start=(k == 0), stop=(k == KD - 1),
                                )
                    for i in range(8):
                        ht = hpool.tile([P, MM_FREE], bf16, tag="hT", name=f"hT_0_{i}")
                        nc.scalar.activation(
                            ht[:, :gs], pss[i][:, :gs],
                            mybir.ActivationFunctionType.Silu,
                        )
                        hts.append(ht)
                    irange = range(8, KH)
                else:
                    irange = range(KH)
                for i in irange:
                    ci, co = divmod(i * P, MM_FREE)
                    ps = psum1.tile([P, MM_FREE], fp32, tag="ps1", name=f"ps1_{gi}_{i}")
                    for k in range(KD):
                        nc.tensor.matmul(
                            ps[:, :gs], w1_view(k, ci, co), xT_view(k, gi),
                            start=(k == 0), stop=(k == KD - 1),
                        )
                    ht = hpool.tile([P, MM_FREE], bf16, tag="hT", name=f"hT_{gi}_{i}")
                    nc.scalar.activation(
                        ht[:, :gs], ps[:, :gs], mybir.ActivationFunctionType.Silu
                    )
                    hts.append(ht)
                return hts

            def mm2(gi, hts):
                t0, gs = goff[gi], gsizes[gi]
                for t in range(gs // P):
                    tok = t0 + t * P
                    for j in range(NJ):
                        ps2 = psum2.tile([P, MM_FREE], fp32, tag="ps2",
                                         name=f"ps2_{tok}_{j}")
                        for i in range(KH):
                            nc.tensor.matmul(
                                ps2,
                                hts[i][:, t * P:(t + 1) * P],
                                w2_view(i, j),
                                start=(i == 0), stop=(i == KH - 1),
                            )
                        ot = opool.tile([P, MM_FREE], fp32, tag="ot",
                                        name=f"ot_{tok}_{j}")
                        nc.vector.tensor_scalar_mul(
                            ot, ps2, wt_sb[:, tok // P:tok // P + 1]
                        )
                        nc.sync.dma_start(
                            out=out[tok:tok + P, j * MM_FREE:(j + 1) * MM_FREE], in_=ot
                        )

            prev = (0, mm1(0))
            for gi in range(1, len(gsizes)):
                hts = mm1(gi)
                mm2(*prev)
                prev = (gi, hts)
            mm2(*prev)

    nc.compile()
    return nc


def _get_compiled(C):
    if C not in _compiled:
        _compiled[C] = _build(C)
    return _compiled[C]


def _pack_xT(xTe, g0, n12, n34):
    """Split x^T [D, C] into the k-major SBUF-image layouts."""
    arr = np.ascontiguousarray(xTe).reshape(KD, P, xTe.shape[1])
    m = {"xg0": np.ascontiguousarray(
        arr[:, :, :g0].transpose(1, 0, 2).reshape(P, KD * g0))}
    if n12:
        m["xg12"] = np.ascontiguousarray(
            arr[:, :, g0:g0 + n12].transpose(1, 0, 2).reshape(P, KD * n12))
    if n34:
        m["xg34"] = np.ascontiguousarray(
            arr[:, :, g0 + n12:].transpose(1, 0, 2).reshape(P, KD * n34))
    return m


def kernel(**inputs):
    x = np.asarray(inputs["x"], dtype=np.float32)
    Wg = np.asarray(inputs["Wg"], dtype=np.float32)
    W1 = np.asarray(inputs["W1"], dtype=np.float32)
    W2 = np.asarray(inputs["W2"], dtype=np.float32)
    xf = np.ascontiguousarray(x.reshape(-1, D))

    # --- host-side gate + top-2 routing (float64; ordering matches f32 ref) ---
    logits = xf.astype(np.float64) @ Wg.astype(np.float64)
    w = np.exp(logits - logits.max(axis=-1, keepdims=True))
    w /= w.sum(axis=-1, keepdims=True)
    order = np.argsort(-w, axis=-1, kind="stable")[:, :TOP_K]  # [N, 2] expert ids
    tw = np.take_along_axis(w, order, axis=-1)
    tw = tw / tw.sum(axis=-1, keepdims=True)  # renormalized combine weights

    counts = np.bincount(order.ravel(), minlength=E)
    C = int(-(-max(int(counts.max()), 512) // P) * P)
    # per-partition SBUF: xT images 16*C bytes + ~107KB of weights/pools
    assert 16 * C + 110 * 1024 < 200 * 1024, "pathological routing skew"

    nc = _get_compiled(C)
    gsizes = _plan_groups(C)
    g0 = gsizes[0]
    n12 = sum(gsizes[1:3]) if len(gsizes) > 1 else 0
    n34 = C - g0 - n12

    bf = ml_dtypes.bfloat16
    pos = np.empty((N, TOP_K), dtype=np.int64)
    in_maps = []
    for e in range(E):
        sel = np.nonzero((order == e).any(axis=-1))[0]
        slot = (order[sel, 1] == e).astype(np.int64)
        pos[sel, slot] = e * C + np.arange(len(sel))

        xTe = np.zeros((D, C), dtype=bf)
        xTe[:, :len(sel)] = xf[sel].T.astype(bf)
        wtp = np.zeros(C, dtype=np.float32)
        wtp[:len(sel)] = tw[sel, slot].astype(np.float32)

        m = _pack_xT(xTe, g0, n12, n34)
        W1e = np.ascontiguousarray(W1[e]).astype(bf).reshape(KD, P, NW1C, MM_FREE)
        for c in range(NW1C):
            m[f"w1c{c}"] = np.ascontiguousarray(
                W1e[:, :, c, :].transpose(1, 0, 2).reshape(P, KD * MM_FREE))
        m["w2i"] = np.ascontiguousarray(
            np.ascontiguousarray(W2[e]).astype(bf).reshape(KH, P, D)
            .transpose(1, 0, 2).reshape(P, KH * D))
        m["wt"] = np.ascontiguousarray(wtp.reshape(C // P, P).T)
        in_maps.append(m)

    from concourse.bass_utils import run_bass_kernel_spmd

    # The SPMD launch reaches the 8 NeuronCores through jax/PJRT. If the
    # calling process pinned jax to CPU (e.g. to run the reference), flip to
    # the axon platform for the launch and restore afterwards.
    import jax

    flipped = False
    try:
        n_acc = len([d for d in jax.devices() if d.platform != "cpu"])
    except Exception:
        n_acc = 0

    def _clear_backends():
        try:
            import jax.extend.backend as jeb
            jeb.clear_backends()
        except Exception:
            from jax._src import xla_bridge
            xla_bridge._clear_backends()

    if n_acc < NCORES:
        prev = jax.config.jax_platforms
        jax.config.update("jax_platforms", "axon")
        _clear_backends()
        flipped = True
    try:
        res = run_bass_kernel_spmd(nc, in_maps, core_ids=list(range(NCORES)))
    finally:
        if flipped:
            jax.config.update("jax_platforms", prev)
            _clear_backends()

    Y = np.concatenate([res.results[c]["out"] for c in range(NCORES)], axis=0)
    outf = Y[pos[:, 0]] + Y[pos[:, 1]]
    return outf.reshape(B, T, D).astype(np.float32)
